# revision 46
# baseline (speedup 1.0000x reference)
"""Trainium2 Bass kernel for nn_Nonlocal (sparse_attention, non-local style attention).

Math (per batch b):
  xn  = instance_norm(content);  sn = instance_norm(style)
  Th  = theta_w @ xn + theta_b          (256, 4096)
  Ph  = phi_w   @ sn + phi_b            (256, 4096)
  g   = g_w @ fusion_style + g_b        (256, 4096)
  f[l,m] = sum_k scale[k]^2 * <Th[:, N_k(l)], Ph[:, N_k(m)]>   (4096, 4096)
           where N_k = 3x3 reflect-padded neighborhood shift
  P = softmax_rows(f);  y = P @ g^T;  out = W_w @ y^T + W_b    (512, 4096)

Sharding: 8 cores = 2 batches x 4 query-row shards (1024 rows of f each).
Instance-norm is folded into the conv weights on the host. The 3x3 shifts are
folded into matmul access patterns: j-axis (within-64 with reflection) via
materialized shifted copies of Th/Ph, i-axis (+-64) via column offsets over
reflect-extended windows. Softmax is computed flash-style over four
1024-column PSUM quarters. PV uses PE transposes of P and fp16 matmuls.

Wire format: the run is transfer-bound (axon tunnel ~70 MB/s up, shared), so
every per-batch tensor (content, style, fusion, folded weights) is uploaded
quarter-split -- each core gets 1/4 -- and reassembled on-device with one
AllGather per 4-core batch group. Content/style travel as fp16 (the f logits
have sigma~96 and softmax is near-argmax, so coarser wire formats blow the
error budget); fusion travels as per-channel-scaled int8 (it only enters the
output linearly through P@g). The output returns as per-channel int8 with an
f32 scale packed into the last 4 bytes of each row. Theta's query window is
sliced on-device from the gathered content with a partition-id-derived
dynamic offset. Host packing is ordered so the cheap tensors hit the tunnel
first and the stats/weight folding overlaps the streaming; the PJRT executor
is built once and cached, and each call donates the previous call's output
buffers back to the kernel.
"""
import numpy as np

import jax
import jax.numpy as jnp
from jax.sharding import Mesh, PartitionSpec, NamedSharding
from jax.experimental.shard_map import shard_map

import concourse.mybir as mybir
from concourse import bacc, bass2jax
from concourse.bass import ds
from concourse.tile import TileContext
from concourse.masks import make_identity

F32 = mybir.dt.float32
F16 = mybir.dt.float16
I8 = mybir.dt.int8

B, C, H, Wd = 2, 512, 64, 64
HW = H * Wd          # 4096
IC = 256
L = HW // 4          # 1024 query rows per core
WIN = L + 2 * 64     # 1152 theta window cols
EXT = HW + 2 * 64    # 4224 phi extended cols
NT = L // 128        # 8 tiles per core
NQ = 4               # psum quarters per tile (1024 cols each)
QC = HW // NQ        # 1024
N_CORES = 8
GROUPS = [[0, 1, 2, 3], [4, 5, 6, 7]]


def _build_program():
    nc = bacc.Bacc("TRN2", target_bir_lowering=False, debug=False,
                   num_devices=N_CORES)

    # style quarter fp16 (c-groups); fusion quarter int8 bitcast-packed
    # (slab k holds c-groups 2k,2k+1 as int8 halves)
    sb_d = nc.dram_tensor("sb", [4, 128, 1024], F16, kind="ExternalInput")
    fb_d = nc.dram_tensor("fb", [2, 128, 1024], F16, kind="ExternalInput")
    wq_d = nc.dram_tensor("wq", [128, 1024], F16, kind="ExternalInput")
    cb_d = nc.dram_tensor("cb", [4, 128, 1024], F16, kind="ExternalInput")
    # small rows: 0-1 bth, 2-3 bph, 4-7 bout, 8-11 fusion dequant scale
    small_d = nc.dram_tensor("small", [12, 128, 1], F32, kind="ExternalInput")
    # int8 output; last 4 cols of each row hold the f32 scale (bitcast)
    o_d = nc.dram_tensor("o", [4, 128, L + 4], I8, kind="ExternalOutput")

    with TileContext(nc) as tc:
        with tc.tile_pool(name="dram", bufs=1, space="DRAM") as dpool, \
             tc.tile_pool(name="const", bufs=1) as constp, \
             tc.tile_pool(name="persist", bufs=1) as persist, \
             tc.tile_pool(name="work", bufs=2) as work, \
             tc.tile_pool(name="stats", bufs=3) as stats, \
             tc.tile_pool(name="fqp", bufs=2, space="PSUM") as fqp, \
             tc.tile_pool(name="ptp", bufs=2, space="PSUM") as ptp, \
             tc.tile_pool(name="yp", bufs=2, space="PSUM") as yp:

            # ---- gather per-batch quarters into full tensors ----
            sbounce = dpool.tile([4, 128, 1024], F16)
            sgath = dpool.tile([4, 4, 128, 1024], F16)
            fbounce = dpool.tile([2, 128, 1024], F16)
            fgath = dpool.tile([4, 2, 128, 1024], F16)
            wbounce = dpool.tile([128, 1024], F16)
            wgath = dpool.tile([4, 128, 1024], F16)
            cbounce = dpool.tile([4, 128, 1024], F16)
            cgath = dpool.tile([4, 4, 128, 1024], F16)
            stext = dpool.tile([4, 128, EXT], F16)
            cext = dpool.tile([4, 128, EXT], F16)
            nc.gpsimd.dma_start(sbounce[:], sb_d[:])
            nc.gpsimd.collective_compute(
                "AllGather", mybir.AluOpType.bypass, replica_groups=GROUPS,
                ins=[sbounce.opt()], outs=[sgath.opt()])
            nc.gpsimd.dma_start(fbounce[:], fb_d[:])
            nc.gpsimd.collective_compute(
                "AllGather", mybir.AluOpType.bypass, replica_groups=GROUPS,
                ins=[fbounce.opt()], outs=[fgath.opt()])
            nc.gpsimd.dma_start(wbounce[:], wq_d[:])
            nc.gpsimd.collective_compute(
                "AllGather", mybir.AluOpType.bypass, replica_groups=GROUPS,
                ins=[wbounce.opt()], outs=[wgath.opt()])
            nc.gpsimd.dma_start(cbounce[:], cb_d[:])
            nc.gpsimd.collective_compute(
                "AllGather", mybir.AluOpType.bypass, replica_groups=GROUPS,
                ins=[cbounce.opt()], outs=[cgath.opt()])
            # style/content reflect-extended windows in DRAM
            for gat, ext in ((sgath, stext), (cgath, cext)):
                for s in range(4):
                    nc.sync.dma_start(
                        out=ext[:, :, 64 + 1024 * s:64 + 1024 * (s + 1)],
                        in_=gat[s, :, :, :])
                nc.sync.dma_start(out=ext[:, :, 0:64], in_=gat[0, :, :, 64:128])
                nc.sync.dma_start(out=ext[:, :, EXT - 64:EXT],
                                  in_=gat[3, :, :, 896:960])

            ident = constp.tile([128, 128], F32)
            make_identity(nc, ident)
            thw = constp.tile([128, 4, IC], F16)
            phw = constp.tile([128, 4, IC], F16)
            gw = constp.tile([128, 4, IC], F16)
            ww = constp.tile([128, 2, C], F16)
            bth = constp.tile([128, 2, 1], F32)
            bph = constp.tile([128, 2, 1], F32)
            bout = constp.tile([128, 4, 1], F32)
            fsc = constp.tile([128, 4, 1], F32)
            for kk in range(4):
                nc.sync.dma_start(out=thw[:, kk, :], in_=wgath[kk, :, 0:256])
                nc.sync.dma_start(out=phw[:, kk, :], in_=wgath[kk, :, 256:512])
                nc.sync.dma_start(out=gw[:, kk, :], in_=wgath[kk, :, 512:768])
                nc.sync.dma_start(
                    out=ww[:, kk // 2, 256 * (kk % 2):256 * (kk % 2) + 256],
                    in_=wgath[kk, :, 768:1024])
            nc.sync.dma_start(out=bth[:, :, :], in_=small_d[0:2].transpose([1, 0, 2]))
            nc.sync.dma_start(out=bph[:, :, :], in_=small_d[2:4].transpose([1, 0, 2]))
            nc.sync.dma_start(out=bout[:, :, :], in_=small_d[4:8].transpose([1, 0, 2]))
            nc.sync.dma_start(out=fsc[:, :, :], in_=small_d[8:12].transpose([1, 0, 2]))

            # persistent big tensors
            th_j = persist.tile([128, 2, 3, WIN], F16)     # theta, j-shifted x3
            ph_j = persist.tile([128, 2, 3, EXT], F16)     # phi, j-shifted x3
            gt = persist.tile([128, 32, IC], F16)          # g^T chunks (m-part)
            yT = persist.tile([128, 2, L], F16)            # y^T accumulator

            # ---- stage A: gT from fusion_style (int8 -> fp16 dequant) ----
            with tc.tile_pool(name="ful", bufs=1) as ful:
                for mg in range(4):
                    raw = ful.tile([128, 2, 1024], F16, tag="furaw")
                    for k2 in range(2):
                        nc.sync.dma_start(out=raw[:, k2, :],
                                          in_=fgath[mg, k2, :, :])
                    fu = ful.tile([128, 4, 1024], F16, tag="fu")
                    for kk in range(4):
                        src = raw[:, kk // 2, :].bitcast(I8)[
                            :, 1024 * (kk % 2):1024 * (kk % 2) + 1024]
                        nc.vector.tensor_scalar_mul(fu[:, kk, :], src,
                                                    fsc[:, kk, :])
                    for jj in range(8):
                        j = 8 * mg + jj
                        ps = yp.tile([128, IC], F32, tag="yps")
                        for kk in range(4):
                            nc.tensor.matmul(ps, fu[:, kk, 128 * jj:128 * (jj + 1)],
                                             gw[:, kk, :], start=(kk == 0), stop=(kk == 3))
                        if j % 2 == 0:
                            nc.vector.tensor_copy(gt[:, j, :], ps)
                        else:
                            nc.scalar.copy(gt[:, j, :], ps)

            # ---- stage B: phi (with bias) and its j-shifted copies ----
            with tc.tile_pool(name="stl", bufs=1) as stl:
                for mg in range(5):
                    g0 = 1024 * mg
                    gw_cols = min(1024, EXT - g0)
                    st = stl.tile([128, 4, 1024], F16, tag="st")
                    for kk in range(4):
                        nc.sync.dma_start(out=st[:, kk, 0:gw_cols],
                                          in_=stext[kk, :, g0:g0 + gw_cols])
                    for oc in range(2):
                        n0 = 0
                        while n0 < gw_cols:
                            nn = min(512, gw_cols - n0)
                            ps = fqp.tile([128, 1024], F32, tag="fq")
                            for kk in range(4):
                                nc.tensor.matmul(ps[:, 0:nn],
                                                 phw[:, kk, 128 * oc:128 * (oc + 1)],
                                                 st[:, kk, n0:n0 + nn],
                                                 start=(kk == 0), stop=(kk == 3))
                            nc.vector.tensor_scalar_add(
                                ph_j[:, oc, 1, g0 + n0:g0 + n0 + nn],
                                ps[:, 0:nn], bph[:, oc, :])
                            n0 += nn
                # j-shifted copies (within 64-col blocks, reflect at edges)
                for oc in range(2):
                    src = ph_j[:, oc, 1, :].rearrange("p (b j) -> p b j", j=64)
                    for dj, dst_i in ((0, 0), (2, 2)):
                        dst = ph_j[:, oc, dst_i, :].rearrange("p (b j) -> p b j", j=64)
                        if dj == 0:
                            nc.vector.tensor_copy(dst[:, :, 1:64], src[:, :, 0:63])
                            nc.scalar.copy(dst[:, :, 0:1], src[:, :, 1:2])
                        else:
                            nc.vector.tensor_copy(dst[:, :, 0:63], src[:, :, 1:64])
                            nc.scalar.copy(dst[:, :, 63:64], src[:, :, 62:63])

            # ---- stage C: theta (with bias) and its j-shifted copies ----
            # this core's query window = ext cols [1024*sh, 1024*sh + WIN)
            pid = nc.sync.partition_id()
            q0 = (pid % 4) * 1024
            with tc.tile_pool(name="cwl", bufs=1) as cwl:
                for mg in range(2):
                    g0 = 1024 * mg
                    gw_cols = min(1024, WIN - g0)
                    cwt = cwl.tile([128, 4, 1024], F16, tag="cwt")
                    for kk in range(4):
                        nc.sync.dma_start(out=cwt[:, kk, 0:gw_cols],
                                          in_=cext[kk, :, ds(q0 + g0, gw_cols)])
                    for oc in range(2):
                        n0 = 0
                        while n0 < gw_cols:
                            nn = min(512, gw_cols - n0)
                            ps = fqp.tile([128, 1024], F32, tag="fq")
                            for kk in range(4):
                                nc.tensor.matmul(ps[:, 0:nn],
                                                 thw[:, kk, 128 * oc:128 * (oc + 1)],
                                                 cwt[:, kk, n0:n0 + nn],
                                                 start=(kk == 0), stop=(kk == 3))
                            nc.vector.tensor_scalar_add(
                                th_j[:, oc, 1, g0 + n0:g0 + n0 + nn],
                                ps[:, 0:nn], bth[:, oc, :])
                            n0 += nn
                for oc in range(2):
                    src = th_j[:, oc, 1, :].rearrange("p (b j) -> p b j", j=64)
                    for dj, dst_i in ((0, 0), (2, 2)):
                        dst = th_j[:, oc, dst_i, :].rearrange("p (b j) -> p b j", j=64)
                        if dj == 0:
                            nc.vector.tensor_copy(dst[:, :, 1:64], src[:, :, 0:63])
                            nc.scalar.copy(dst[:, :, 0:1], src[:, :, 1:2])
                        else:
                            nc.vector.tensor_copy(dst[:, :, 0:63], src[:, :, 1:64])
                            nc.scalar.copy(dst[:, :, 63:64], src[:, :, 62:63])

            # ---- main loop over 8 query tiles ----
            for t in range(NT):
                negM = stats.tile([128, 1], F32, tag="negM")
                s_run = stats.tile([128, 1], F32, tag="s_run")
                y_sb = work.tile([128, IC], F32, tag="y_sb")
                for q in range(NQ):
                    fq = fqp.tile([128, QC], F32, tag="fq")
                    for nn in range(2):
                        cs = slice(512 * nn, 512 * (nn + 1))
                        first = True
                        for dj in range(3):
                            for di in range(3):
                                for cc in range(2):
                                    last = (dj == 2 and di == 2 and cc == 1)
                                    nc.tensor.matmul(
                                        fq[:, cs],
                                        th_j[:, cc, dj, 128 * t + 64 * di:
                                             128 * t + 64 * di + 128],
                                        ph_j[:, cc, dj, 64 * di + QC * q + 512 * nn:
                                             64 * di + QC * q + 512 * (nn + 1)],
                                        start=first, stop=last)
                                    first = False
                    # flash-style softmax over quarters
                    negmq = stats.tile([128, 1], F32, tag="negmq")
                    nc.vector.tensor_reduce(negmq, fq, axis=mybir.AxisListType.X,
                                            op=mybir.AluOpType.max, negate=True)
                    sq = stats.tile([128, 1], F32, tag="sq")
                    pq = work.tile([128, QC], F32, tag="pq")
                    if q == 0:
                        nc.vector.tensor_copy(negM, negmq)
                        nc.scalar.activation(pq, fq, mybir.ActivationFunctionType.Exp,
                                             bias=negM, scale=1.0, accum_out=s_run)
                    else:
                        posM_old = stats.tile([128, 1], F32, tag="posM")
                        nc.vector.tensor_scalar_mul(posM_old, negM, -1.0)
                        nc.vector.tensor_tensor(negM, negM, negmq,
                                                op=mybir.AluOpType.min)
                        cfac = stats.tile([128, 1], F32, tag="cfac")
                        nc.scalar.activation(cfac, negM,
                                             mybir.ActivationFunctionType.Exp,
                                             bias=posM_old, scale=1.0)
                        nc.scalar.activation(pq, fq, mybir.ActivationFunctionType.Exp,
                                             bias=negM, scale=1.0, accum_out=sq)
                        nc.vector.tensor_scalar_mul(s_run, s_run, cfac)
                        nc.vector.tensor_tensor(s_run, s_run, sq,
                                                op=mybir.AluOpType.add)
                        nc.vector.tensor_scalar_mul(y_sb, y_sb, cfac)
                    # transpose P quarter + PV partial
                    y_ps = yp.tile([128, IC], F32, tag="yps")
                    ptsb = work.tile([128, 8, 128], F16, tag="ptsb")
                    for j in range(8):
                        pt_ps = ptp.tile([128, 128], F32, tag="pt")
                        nc.tensor.transpose(pt_ps, pq[:, 128 * j:128 * (j + 1)], ident)
                        if j % 2 == 0:
                            nc.vector.tensor_copy(ptsb[:, j, :], pt_ps)
                        else:
                            nc.scalar.copy(ptsb[:, j, :], pt_ps)
                    for j in range(8):
                        nc.tensor.matmul(y_ps, ptsb[:, j, :], gt[:, 8 * q + j, :],
                                         start=(j == 0), stop=(j == 7))
                    if q == 0:
                        nc.vector.tensor_copy(y_sb, y_ps)
                    else:
                        nc.vector.tensor_tensor(y_sb, y_sb, y_ps,
                                                op=mybir.AluOpType.add)
                # normalize and transpose y into yT
                rec = stats.tile([128, 1], F32, tag="rec")
                nc.vector.reciprocal(rec, s_run)
                yn = work.tile([128, IC], F32, tag="yn")
                nc.vector.tensor_scalar_mul(yn, y_sb, rec)
                for oc in range(2):
                    yt_ps = ptp.tile([128, 128], F32, tag="pt")
                    nc.tensor.transpose(yt_ps, yn[:, 128 * oc:128 * (oc + 1)], ident)
                    nc.vector.tensor_copy(yT[:, oc, 128 * t:128 * (t + 1)], yt_ps)

            # ---- tail: W conv + bias + int8 quantize + store ----
            with tc.tile_pool(name="outp", bufs=2) as outp:
                for mo in range(4):
                    ot = outp.tile([128, L], F32, tag="ot")
                    oabs = outp.tile([128, L], F32, tag="oabs")
                    for nl in range(2):
                        ps = fqp.tile([128, QC], F32, tag="fq")
                        for kk in range(2):
                            nc.tensor.matmul(ps[:, 0:512],
                                             ww[:, kk, 128 * mo:128 * (mo + 1)],
                                             yT[:, kk, 512 * nl:512 * (nl + 1)],
                                             start=(kk == 0), stop=(kk == 1))
                        nc.scalar.activation(ot[:, 512 * nl:512 * (nl + 1)],
                                             ps[:, 0:512],
                                             mybir.ActivationFunctionType.Identity,
                                             bias=bout[:, mo, :], scale=1.0)
                    nc.scalar.activation(oabs, ot,
                                         mybir.ActivationFunctionType.Abs,
                                         scale=1.0)
                    amax = stats.tile([128, 1], F32, tag="amax")
                    nc.vector.tensor_reduce(amax, oabs, axis=mybir.AxisListType.X,
                                            op=mybir.AluOpType.max)
                    nc.vector.tensor_scalar_add(amax, amax, 1e-20)
                    rsc = stats.tile([128, 1], F32, tag="rsc")
                    nc.vector.reciprocal(rsc, amax)
                    nc.vector.tensor_scalar_mul(rsc, rsc, 127.0)
                    osc = stats.tile([128, 1], F32, tag="osc")
                    nc.vector.tensor_scalar_mul(osc, amax, 1.0 / 127.0)
                    oq = outp.tile([128, L], I8, tag="oq")
                    nc.vector.tensor_scalar_mul(oq, ot, rsc)
                    nc.sync.dma_start(out=o_d[mo][:, 0:L], in_=oq)
                    nc.sync.dma_start(out=o_d[mo][:, L:L + 4].bitcast(F32), in_=osc)

    nc.compile()
    return nc


class _Runner:
    """Persistent PJRT executor: jit built once, zeros created on device."""

    def __init__(self, nc):
        bass2jax.install_neuronx_cc_hook()
        partition_name = (nc.partition_id_tensor.name
                          if nc.partition_id_tensor else None)
        in_names, out_names, out_avals = [], [], []
        for alloc in nc.m.functions[0].allocations:
            if not isinstance(alloc, mybir.MemoryLocationSet):
                continue
            name = alloc.memorylocations[0].name
            if alloc.kind == "ExternalInput":
                if name != partition_name:
                    in_names.append(name)
            elif alloc.kind == "ExternalOutput":
                out_names.append(name)
                out_avals.append(jax.core.ShapedArray(
                    tuple(alloc.tensor_shape), mybir.dt.np(alloc.dtype)))
        all_in = list(in_names) + list(out_names)
        if partition_name is not None:
            all_in.append(partition_name)
        n_params = len(in_names)
        n_outs = len(out_avals)
        donate = tuple(range(n_params, n_params + n_outs))

        def _body(*args):
            operands = list(args)
            if partition_name is not None:
                operands.append(bass2jax.partition_id_tensor())
            return tuple(bass2jax._bass_exec_p.bind(
                *operands, out_avals=tuple(out_avals), in_names=tuple(all_in),
                out_names=tuple(out_names), lowering_input_output_aliases=(),
                sim_require_finite=True, sim_require_nnan=True, nc=nc))

        devices = jax.devices()[:N_CORES]
        mesh = Mesh(np.asarray(devices), ("core",))
        spec = PartitionSpec("core")
        self.sharding = NamedSharding(mesh, spec)
        self.fn = jax.jit(
            shard_map(_body, mesh=mesh,
                      in_specs=(spec,) * (n_params + n_outs),
                      out_specs=(spec,) * n_outs, check_rep=False),
            donate_argnums=donate, keep_unused=True)
        zero_shapes = [(N_CORES * a.shape[0], *a.shape[1:]) for a in out_avals]
        zero_dtypes = [a.dtype for a in out_avals]
        self.mkzeros = jax.jit(
            lambda: tuple(jnp.zeros(s, d)
                          for s, d in zip(zero_shapes, zero_dtypes)),
            out_shardings=(self.sharding,) * n_outs)
        self.in_names = in_names
        self.out_names = out_names
        self.out_avals = out_avals
        self._donate_next = None

    def __call__(self, inputs):
        # recycle the previous call's (fully fetched) output buffers as the
        # donated output params; the kernel writes every output element.
        donate = self._donate_next or self.mkzeros()
        self._donate_next = None
        outs = self.fn(*[inputs[n] for n in self.in_names], *donate)
        for a in outs:
            a.copy_to_host_async()
        res = {n: np.asarray(a) for n, a in zip(self.out_names, outs)}
        self._donate_next = outs
        return res


_PROG = None
_RUNNER = None


def _pack_s(style):
    """Style quarters (fp16) -- cheapest pack, uploaded first."""
    sf = style.reshape(B, C, HW)
    s_all = np.empty((N_CORES, 4, 128, 1024), np.float16)
    for b in range(B):
        sf4 = sf[b].reshape(4, 128, HW)
        for sh in range(4):
            s_all[4 * b + sh] = sf4[:, :, 1024 * sh:1024 * (sh + 1)]
    return s_all.reshape(N_CORES * 4, 128, 1024)


def _pack_f(fusion):
    """Fusion quarters (int8, per-channel scale), bitcast-packed into fp16."""
    ff = fusion.reshape(B, C, HW)
    amax = np.abs(ff).max(-1)
    amax[amax == 0] = 1.0
    fsc_all = (amax / 127.0).astype(np.float32)
    f8 = np.rint(ff * (127.0 / amax)[:, :, None]).astype(np.int8)
    # per-core slab layout [b][sh][k2][p][j*1024+col] = f8[b][2*k2+j][p][sh*1024+col]
    f8v = f8.reshape(B, 2, 2, 128, 4, 1024)
    f_all = np.ascontiguousarray(f8v.transpose(0, 4, 1, 3, 2, 5))
    return f_all.view(np.float16).reshape(N_CORES * 2, 128, 1024), fsc_all


def _pack_c(content):
    """Content quarters (fp16) -- windows are sliced on device."""
    cf = content.reshape(B, C, HW)
    c_all = np.empty((N_CORES, 4, 128, 1024), np.float16)
    for b in range(B):
        cf4 = cf[b].reshape(4, 128, HW)
        for sh in range(4):
            c_all[4 * b + sh] = cf4[:, :, 1024 * sh:1024 * (sh + 1)]
    return c_all.reshape(N_CORES * 4, 128, 1024)


def _pack_rest(inputs, fsc_all):
    EPS = 1e-5
    content = np.asarray(inputs["content"], np.float32)
    style = np.asarray(inputs["style"], np.float32)
    theta_w = np.asarray(inputs["theta_w"], np.float32)
    theta_b = np.asarray(inputs["theta_b"], np.float32)
    phi_w = np.asarray(inputs["phi_w"], np.float32)
    phi_b = np.asarray(inputs["phi_b"], np.float32)
    g_w = np.asarray(inputs["g_w"], np.float32)
    g_b = np.asarray(inputs["g_b"], np.float32)
    W_w = np.asarray(inputs["W_w"], np.float32)
    W_b = np.asarray(inputs["W_b"], np.float32)
    scale = np.asarray(inputs["scale"], np.float32)

    s2 = scale.astype(np.float64) ** 2
    if not np.allclose(s2, s2[0]):
        raise NotImplementedError("non-uniform ContextAtten scale not supported")
    s0 = float(s2[0])

    cf = content.reshape(B, C, HW)
    sf = style.reshape(B, C, HW)

    def stats(x):
        s1 = x.sum(-1)
        s2 = np.einsum('bch,bch->bc', x, x)
        mu = s1 / HW
        var = (s2 - HW * mu * mu) / (HW - 1)
        return mu, 1.0 / np.sqrt(var + EPS)

    mu_c, rstd_c = stats(cf)
    mu_s, rstd_s = stats(sf)

    gw_t = np.ascontiguousarray(g_w.T).astype(np.float16).reshape(4, 128, IC)
    ww_t = np.ascontiguousarray(W_w.T).astype(np.float16).reshape(2, 128, C)
    bout = (W_w @ g_b + W_b).astype(np.float32).reshape(4, 128, 1)

    wq_all = np.empty((N_CORES, 128, 1024), np.float16)
    small_all = np.empty((N_CORES, 12, 128, 1), np.float32)

    for b in range(B):
        thw = (theta_w * rstd_c[b][None, :] * s0).T.astype(np.float16)
        bth = ((theta_b - theta_w @ (mu_c[b] * rstd_c[b])) * s0).astype(
            np.float32).reshape(2, 128, 1)
        phw = (phi_w * rstd_s[b][None, :]).T.astype(np.float16)
        bph = (phi_b - phi_w @ (mu_s[b] * rstd_s[b])).astype(
            np.float32).reshape(2, 128, 1)
        thw4 = thw.reshape(4, 128, IC)
        phw4 = phw.reshape(4, 128, IC)
        small = np.empty((12, 128, 1), np.float32)
        small[0:2] = bth
        small[2:4] = bph
        small[4:8] = bout
        small[8:12] = fsc_all[b].reshape(4, 128, 1)
        for sh in range(4):
            r = 4 * b + sh
            wslab = wq_all[r]
            wslab[:, 0:256] = thw4[sh]
            wslab[:, 256:512] = phw4[sh]
            wslab[:, 512:768] = gw_t[sh]
            wslab[:, 768:1024] = ww_t[sh // 2][:, 256 * (sh % 2):256 * (sh % 2) + 256]
            small_all[r] = small

    return {
        "wq": wq_all,
        "small": small_all.reshape(N_CORES * 12, 128, 1),
    }


def kernel(**inputs):
    global _PROG, _RUNNER
    if _RUNNER is None:
        _PROG = _build_program()
        _RUNNER = _Runner(_PROG)
    # pack + enqueue cheap tensors first: their upload streams over the
    # tunnel while the host quantizes fusion and folds weights below.
    style = np.asarray(inputs["style"], np.float32)
    fusion = np.asarray(inputs["fusion_style"], np.float32)
    content = np.asarray(inputs["content"], np.float32)
    arrs = {"sb": jax.device_put(_pack_s(style), _RUNNER.sharding)}
    fb, fsc_all = _pack_f(fusion)
    arrs["fb"] = jax.device_put(fb, _RUNNER.sharding)
    arrs["cb"] = jax.device_put(_pack_c(content), _RUNNER.sharding)
    rest = _pack_rest(inputs, fsc_all)
    for n in ("wq", "small"):
        arrs[n] = jax.device_put(rest[n], _RUNNER.sharding)
    res = _RUNNER(arrs)
    raw = res["o"].reshape(N_CORES, 4, 128, L + 4)
    o = raw[:, :, :, 0:L].astype(np.float32)
    o *= np.ascontiguousarray(raw[:, :, :, L:L + 4]).view(np.float32)
    out = np.empty((B, C, HW), np.float32)
    for r in range(N_CORES):
        b, sh = divmod(r, 4)
        out[b][:, sh * L:(sh + 1) * L] = o[r].reshape(C, L)
    return out.reshape(B, C, H, Wd)
